# revision 1
# baseline (speedup 1.0000x reference)
"""nn_ConvP4 TRN2 Bass kernel: P4 group-equivariant convolution on 8 NeuronCores.

out[b,j,h,w,f] = sum_{k,a,v,c} x[b,(j+k-1)%4, h+a-1, w+v-1, c] * rot90(kernel,j)[a,v,k,c,f]
  x: [8,4,128,128,64] f32, kernel: [3,3,3,64,128] f32 -> out: [8,4,128,128,128] f32

Sharding: data-parallel over batch, one batch item per core (8 cores), kernel
weights replicated. No cross-device communication.

Device algorithm (per core / batch item): for each output group j and each
4-row output quad, accumulate 15 matmuls [K=128, M=128(F)] x [K=128, N=512]
into one PSUM bank:
  - 9 taps (a,v): channels of k=0 (low 64 partitions) and k=1 (high 64) are
    stacked so one K=128 matmul computes both group-depth taps (tensor P0).
  - 3 taps (v):  k=2 taps a=0 (low) / a=1 (high) stacked via a row-shifted
    staging tensor P1.
  - 3 taps (v):  k=2, a=2 as K=128 matmuls with zero weights in the high half
    (a K=64 matmul measures ~2x slower than K=128 on TRN2, so zero-padding
    the contraction is faster than a true K=64 matmul).
All matmuls run in float32r (full-rate fp32 mode of the PE; measured output
error ~1.4e-4 scale-relative for this 1728-term contraction).

Host side stages channel-major, spatially zero-padded tensors so the device
only ever issues dense, large, contiguous-per-partition DMAs, and the output
is produced in [j, f, h, w] layout which the host transposes during unshard.
"""

from contextlib import ExitStack

import numpy as np

import concourse.bacc as bacc
import concourse.tile as tile
from concourse import mybir
from concourse.bass_utils import run_bass_kernel_spmd

N_CORES = 8
B = 8
H = W = 128
CIN = 64
F = 128
HP = 132  # padded rows in staged tensors
WP = 130  # padded cols
NQ = 4  # quads per PSUM group
GROUPS = H // (4 * NQ)

F32 = mybir.dt.float32
F32R = mybir.dt.float32r


def _stage_inputs(x: np.ndarray, kern: np.ndarray):
    """Per-core input maps from full inputs."""
    xt = np.ascontiguousarray(x.transpose(0, 1, 4, 2, 3))  # [b,g,c,h,w]
    P0 = np.zeros((B, 4, 128, HP, WP), np.float32)
    P1 = np.zeros((B, 4, 128, HP, WP), np.float32)
    for t in range(4):
        P0[:, t, 0:64, 1 : H + 1, 1 : W + 1] = xt[:, t]
        P0[:, t, 64:128, 1 : H + 1, 1 : W + 1] = xt[:, (t + 1) % 4]
        P1[:, t, 0:64, 2 : H + 2, 1 : W + 1] = xt[:, t]
        P1[:, t, 64:128, 1 : H + 1, 1 : W + 1] = xt[:, t]

    Wpk = np.zeros((4, 15, 128, F), np.float32)
    for j in range(4):
        Kj = np.rot90(kern, k=j, axes=(0, 1))
        for a in range(3):
            for v in range(3):
                Wpk[j, 3 * a + v, 0:64] = Kj[a, v, 0]
                Wpk[j, 3 * a + v, 64:128] = Kj[a, v, 1]
        for v in range(3):
            Wpk[j, 9 + v, 0:64] = Kj[0, v, 2]
            Wpk[j, 9 + v, 64:128] = Kj[1, v, 2]
            Wpk[j, 12 + v, 0:64] = Kj[2, v, 2]
            # high half of slots 12-14 stays zero
    return [{"p0": P0[b], "p1": P1[b], "wt": Wpk} for b in range(B)]


def build_program(loop_iters: int = 1, out_bufs: int = 6, win_bufs: int = 3):
    """Build + compile the per-core Bass program (identical on all cores)."""
    nc = bacc.Bacc("TRN2", target_bir_lowering=False, debug=False, num_devices=N_CORES)

    p0 = nc.dram_tensor("p0", [4, 128, HP, WP], F32R, kind="ExternalInput").ap()
    p1 = nc.dram_tensor("p1", [4, 128, HP, WP], F32R, kind="ExternalInput").ap()
    wt = nc.dram_tensor("wt", [4, 15, 128, F], F32R, kind="ExternalInput").ap()
    out = nc.dram_tensor("out_t", [4, F, H, W], F32, kind="ExternalOutput").ap()

    rows_per_group = 4 * NQ  # 16
    win_rows = rows_per_group + 2  # 18

    with tile.TileContext(nc) as tc, ExitStack() as ctx:
        wpool = ctx.enter_context(tc.tile_pool(name="wts", bufs=1))
        winpool = ctx.enter_context(tc.tile_pool(name="win", bufs=win_bufs))
        pspool = ctx.enter_context(tc.tile_pool(name="ps", bufs=8, space="PSUM"))
        outpool = ctx.enter_context(tc.tile_pool(name="ob", bufs=out_bufs))

        # All 60 weight tiles resident in SBUF, loaded once.
        w_all = wpool.tile([128, 60 * F], F32R)
        nc.sync.dma_start(
            w_all[:].rearrange("p (s f) -> p s f", f=F),
            wt.rearrange("j s c f -> c (j s) f"),
        )

        def body(_iv=None):
            for j in range(4):
                t0 = (j + 3) % 4
                t1 = (j + 1) % 4
                for g in range(GROUPS):
                    h0 = rows_per_group * g
                    win0 = winpool.tile([128, win_rows * WP], F32R, tag="win0")
                    nc.sync.dma_start(
                        win0[:].rearrange("p (r c) -> p r c", c=WP),
                        p0[t0, :, h0 : h0 + win_rows, :],
                    )
                    win1 = winpool.tile([128, win_rows * WP], F32R, tag="win1")
                    nc.scalar.dma_start(
                        win1[:].rearrange("p (r c) -> p r c", c=WP),
                        p1[t1, :, h0 + 1 : h0 + 1 + win_rows, :],
                    )
                    w0r = win0[:].rearrange("p (r c) -> p r c", c=WP)
                    w1r = win1[:].rearrange("p (r c) -> p r c", c=WP)

                    psums = [
                        pspool.tile([128, 512], F32, tag="ps", name=f"ps_{j}_{g}_{q}")
                        for q in range(NQ)
                    ]
                    for s in range(15):
                        sl = (j * 15 + s) * F
                        for q in range(NQ):
                            r0 = 4 * q
                            if s < 9:
                                a, v = divmod(s, 3)
                                rhs = w0r[:, r0 + a : r0 + a + 4, v : v + W]
                            elif s < 12:
                                v = s - 9
                                rhs = w1r[:, r0 : r0 + 4, v : v + W]
                            else:
                                v = s - 12
                                rhs = w1r[:, r0 + 2 : r0 + 6, v : v + W]
                            nc.tensor.matmul(
                                psums[q][:],
                                w_all[:, sl : sl + F],
                                rhs,
                                start=(s == 0),
                                stop=(s == 14),
                            )
                    for q in range(NQ):
                        ot = outpool.tile([128, 512], F32, tag="ob")
                        nc.vector.tensor_copy(ot[:], psums[q][:])
                        nc.scalar.dma_start(
                            out[j, :, h0 + 4 * q : h0 + 4 * q + 4, :],
                            ot[:].rearrange("p (r c) -> p r c", c=W),
                        )

        if loop_iters > 1:
            with tc.For_i(0, loop_iters, 1) as iv:
                body(iv)
        else:
            body()

    nc.compile()
    return nc


_PROGRAM_CACHE = {}


def _get_program(loop_iters: int = 1):
    if loop_iters not in _PROGRAM_CACHE:
        _PROGRAM_CACHE[loop_iters] = build_program(loop_iters)
    return _PROGRAM_CACHE[loop_iters]


def kernel(**inputs) -> np.ndarray:
    x = np.ascontiguousarray(np.asarray(inputs["x"], dtype=np.float32))
    kern = np.ascontiguousarray(np.asarray(inputs["kernel"], dtype=np.float32))
    assert x.shape == (B, 4, H, W, CIN), x.shape
    assert kern.shape == (3, 3, 3, CIN, F), kern.shape

    nc = _get_program(1)
    in_maps = _stage_inputs(x, kern)

    last_err = None
    for _attempt in range(3):
        try:
            res = run_bass_kernel_spmd(nc, in_maps, list(range(N_CORES)))
            break
        except Exception as e:  # transient device wedge: retry
            last_err = e
    else:
        raise last_err

    stacked = np.stack([r["out_t"] for r in res.results])  # [b, j, f, h, w]
    return np.ascontiguousarray(stacked.transpose(0, 1, 3, 4, 2))



# revision 2
# speedup vs baseline: 2.4492x; 2.4492x over previous
"""nn_ConvP4 TRN2 Bass kernel v2: bf16 inputs, 14 matmul slots per output
group (vs 15), three staged window tensors.

Tap packing per output group j (27 taps x 64ch = 13.5 K=128 slots):
  s=0..8  : (a,v) pairs: low=plane j-1 (k=0), high=plane j (k=1)   [P0]
  s=9..11 : v-taps, k=2 plane j+1: low=a0, high=a1 (row-shift pair) [P1]
  s=12    : k=2 plane j+1, a=2: low=v0, high=v1 (col-shift pair)    [P2]
  s=13    : k=2 plane j+1, (a=2,v=2): low only, high zero           [P1]
"""

from contextlib import ExitStack

import numpy as np
import ml_dtypes

import concourse.bacc as bacc
import concourse.tile as tile
from concourse import mybir
from concourse.bass_utils import run_bass_kernel_spmd

N_CORES = 8
B = 8
H = W = 128
CIN = 64
F = 128
HP = 132
WP = 130
NQ = 4
GROUPS = H // (4 * NQ)
NSLOT = 14

F32 = mybir.dt.float32
BF16 = mybir.dt.bfloat16
BF = ml_dtypes.bfloat16


def _stage_inputs(x: np.ndarray, kern: np.ndarray):
    xt = np.ascontiguousarray(x.transpose(0, 1, 4, 2, 3))  # [b,g,c,h,w]
    P0 = np.zeros((B, 4, 128, HP, WP), BF)
    P1 = np.zeros((B, 4, 128, HP, WP), BF)
    P2 = np.zeros((B, 4, 128, HP, WP), BF)
    xb = xt.astype(BF)
    for t in range(4):
        P0[:, t, 0:64, 1 : H + 1, 1 : W + 1] = xb[:, t]
        P0[:, t, 64:128, 1 : H + 1, 1 : W + 1] = xb[:, (t + 1) % 4]
        P1[:, t, 0:64, 2 : H + 2, 1 : W + 1] = xb[:, t]
        P1[:, t, 64:128, 1 : H + 1, 1 : W + 1] = xb[:, t]
        P2[:, t, 0:64, 2 : H + 2, 2 : W + 2] = xb[:, t]
        P2[:, t, 64:128, 2 : H + 2, 1 : W + 1] = xb[:, t]

    Wpk = np.zeros((4, NSLOT, 128, F), np.float32)
    for j in range(4):
        Kj = np.rot90(kern, k=j, axes=(0, 1))
        for a in range(3):
            for v in range(3):
                Wpk[j, 3 * a + v, 0:64] = Kj[a, v, 0]
                Wpk[j, 3 * a + v, 64:128] = Kj[a, v, 1]
        for v in range(3):
            Wpk[j, 9 + v, 0:64] = Kj[0, v, 2]
            Wpk[j, 9 + v, 64:128] = Kj[1, v, 2]
        Wpk[j, 12, 0:64] = Kj[2, 0, 2]
        Wpk[j, 12, 64:128] = Kj[2, 1, 2]
        Wpk[j, 13, 0:64] = Kj[2, 2, 2]
        # high half of slot 13 stays zero
    Wb = Wpk.astype(BF)
    return [
        {"p0": P0[b], "p1": P1[b], "p2": P2[b], "wt": Wb} for b in range(B)
    ]


def build_program(loop_iters: int = 1, out_bufs: int = 6, win_bufs: int = 3):
    nc = bacc.Bacc("TRN2", target_bir_lowering=False, debug=False, num_devices=N_CORES)

    p0 = nc.dram_tensor("p0", [4, 128, HP, WP], BF16, kind="ExternalInput").ap()
    p1 = nc.dram_tensor("p1", [4, 128, HP, WP], BF16, kind="ExternalInput").ap()
    p2 = nc.dram_tensor("p2", [4, 128, HP, WP], BF16, kind="ExternalInput").ap()
    wt = nc.dram_tensor("wt", [4, NSLOT, 128, F], BF16, kind="ExternalInput").ap()
    out = nc.dram_tensor("out_t", [4, F, H, W], F32, kind="ExternalOutput").ap()

    rows_per_group = 4 * NQ  # 16
    win_rows = rows_per_group + 2  # 18

    with tile.TileContext(nc) as tc, ExitStack() as ctx:
        wpool = ctx.enter_context(tc.tile_pool(name="wts", bufs=1))
        winpool = ctx.enter_context(tc.tile_pool(name="win", bufs=win_bufs))
        pspool = ctx.enter_context(tc.tile_pool(name="ps", bufs=8, space="PSUM"))
        outpool = ctx.enter_context(tc.tile_pool(name="ob", bufs=out_bufs))

        w_all = wpool.tile([128, 4 * NSLOT * F], BF16)
        nc.sync.dma_start(
            w_all[:].rearrange("p (s f) -> p s f", f=F),
            wt.rearrange("j s c f -> c (j s) f"),
        )

        def body(_iv=None):
            for j in range(4):
                t0 = (j + 3) % 4
                t1 = (j + 1) % 4
                for g in range(GROUPS):
                    h0 = rows_per_group * g
                    win0 = winpool.tile([128, win_rows * WP], BF16, tag="win0")
                    nc.sync.dma_start(
                        win0[:].rearrange("p (r c) -> p r c", c=WP),
                        p0[t0, :, h0 : h0 + win_rows, :],
                    )
                    win1 = winpool.tile([128, win_rows * WP], BF16, tag="win1")
                    nc.scalar.dma_start(
                        win1[:].rearrange("p (r c) -> p r c", c=WP),
                        p1[t1, :, h0 + 1 : h0 + 1 + win_rows, :],
                    )
                    win2 = winpool.tile([128, win_rows * WP], BF16, tag="win2")
                    nc.gpsimd.dma_start(
                        win2[:].rearrange("p (r c) -> p r c", c=WP),
                        p2[t1, :, h0 + 1 : h0 + 1 + win_rows, :],
                    )
                    w0r = win0[:].rearrange("p (r c) -> p r c", c=WP)
                    w1r = win1[:].rearrange("p (r c) -> p r c", c=WP)
                    w2r = win2[:].rearrange("p (r c) -> p r c", c=WP)

                    psums = [
                        pspool.tile([128, 512], F32, tag="ps", name=f"ps_{j}_{g}_{q}")
                        for q in range(NQ)
                    ]
                    for s in range(NSLOT):
                        sl = (j * NSLOT + s) * F
                        for q in range(NQ):
                            r0 = 4 * q
                            if s < 9:
                                a, v = divmod(s, 3)
                                rhs = w0r[:, r0 + a : r0 + a + 4, v : v + W]
                            elif s < 12:
                                v = s - 9
                                rhs = w1r[:, r0 : r0 + 4, v : v + W]
                            elif s == 12:
                                rhs = w2r[:, r0 + 2 : r0 + 6, 1 : 1 + W]
                            else:
                                rhs = w1r[:, r0 + 2 : r0 + 6, 2 : 2 + W]
                            nc.tensor.matmul(
                                psums[q][:],
                                w_all[:, sl : sl + F],
                                rhs,
                                start=(s == 0),
                                stop=(s == NSLOT - 1),
                            )
                    for q in range(NQ):
                        ot = outpool.tile([128, 512], F32, tag="ob")
                        nc.vector.tensor_copy(ot[:], psums[q][:])
                        nc.scalar.dma_start(
                            out[j, :, h0 + 4 * q : h0 + 4 * q + 4, :],
                            ot[:].rearrange("p (r c) -> p r c", c=W),
                        )

        if loop_iters > 1:
            with tc.For_i(0, loop_iters, 1) as iv:
                body(iv)
        else:
            body()

    nc.compile()
    return nc


_PROGRAM_CACHE = {}


def _get_program(loop_iters: int = 1):
    if loop_iters not in _PROGRAM_CACHE:
        _PROGRAM_CACHE[loop_iters] = build_program(loop_iters)
    return _PROGRAM_CACHE[loop_iters]


def kernel(**inputs) -> np.ndarray:
    x = np.ascontiguousarray(np.asarray(inputs["x"], dtype=np.float32))
    kern = np.ascontiguousarray(np.asarray(inputs["kernel"], dtype=np.float32))
    assert x.shape == (B, 4, H, W, CIN), x.shape
    assert kern.shape == (3, 3, 3, CIN, F), kern.shape

    nc = _get_program(1)
    in_maps = _stage_inputs(x, kern)

    last_err = None
    for _attempt in range(3):
        try:
            res = run_bass_kernel_spmd(nc, in_maps, list(range(N_CORES)))
            break
        except Exception as e:
            last_err = e
    else:
        raise last_err

    stacked = np.stack([r["out_t"] for r in res.results])  # [b, j, f, h, w]
    return np.ascontiguousarray(stacked.transpose(0, 1, 3, 4, 2))
